# revision 8
# baseline (speedup 1.0000x reference)
"""Trainium2 Bass kernel for a CRF layer (dense matmul potentials + Viterbi decode).

Contract: kernel(**inputs) takes the FULL unsharded inputs (numpy) and returns
(potentials, decoded_onehot), both [64, 512, 128] float32, matching reference().

Strategy (data-parallel over batch, 8 sequences per NeuronCore, SPMD on 8 cores):
  Phase 1 (TensorE): potT[v, tok] = (x @ kernel + bias + boundary)^T via matmuls
    with the bias/boundary folded in as extra contraction rows. potT goes to
    DRAM as-is; the host transposes to [tok, v] (pure layout move).
  Phase 2 (forward Viterbi): replicated-slab layout. Partition p = b*16+vh owns
    the 8 next-states v = vh*8..vh*8+8 of sequence b. Per step:
      - rotation all-gather of the state vector within each 16-partition group
        via 4 stream_shuffles (order seen by partition p: u = (8*vh + j) mod 128)
      - sc = chain_perm + state_row broadcast (optionally split DVE/GpSimd)
      - cmax = max over u (tensor_reduce); backpointers = raw max_index (u16)
        written straight into bp_raw, deferred one step so it overlaps the
        next step's adds; no per-step decode, no bp masking.
      - state = where(mask, cmax + pot_t, state)
  Phase 2b: batched decode of raw indices -> absolute predecessor states,
    done per 64-step chunk right before the backward pass consumes them.
  Phase 3 (backward trace): per step one is_eq (pi == tag) + one
    tensor_tensor_reduce (sel*bp -> sum) + copy_predicated(~mask) to freeze
    tags across padding. Tags land in tag_store[128, T].
  Phase 3b: tag_store -> PE transpose -> 32 bulk is_eq one-hot tiles -> DMA.
"""

import os
import sys

import numpy as np

sys.path.insert(0, "/opt/trn_rl_repo")

from contextlib import ExitStack

import concourse.bacc as bacc
import concourse.bass as bass
import concourse.mybir as mybir
import concourse.tile as tile

B, T, D, U = 64, 512, 1024, 128
NCORES = 8
BL = B // NCORES          # sequences per core
VH, VL = 16, 8            # partition groups of 16; 8 states per partition
TOK = BL * T              # tokens per core
KC = D // 128             # contraction chunks
TCH = 512                 # phase-1 token chunk
CH = 64                   # backward chunk (rows per bp decode/all-gather)

F32 = mybir.dt.float32
U16 = mybir.dt.uint16
U8 = mybir.dt.uint8
ALU = mybir.AluOpType
AX = mybir.AxisListType


def _shuffle_mask(s):
    # quadrant-local: keep the 16-group bit, rotate within the group by s
    return [(i & 16) | ((i + s) & 15) for i in range(32)]


def build_module(t_steps=T, tok=TOK, n_devices=NCORES, add_split=0):
    """Build the SPMD Bass module. t_steps/tok shrinkable for simulation.

    add_split: vl groups [add_split..8) of the per-step sc add run on GpSimd
    (0 = everything on DVE).
    """
    nc = bacc.Bacc(
        "TRN2", target_bir_lowering=False, debug=False, num_devices=n_devices
    )

    tch = min(TCH, tok)
    n_tch = tok // tch
    ch = min(CH, t_steps)
    n_ch = t_steps // ch
    assert t_steps % ch == 0 and tok % tch == 0 and tch % 128 == 0

    # ---- DRAM I/O ----
    x_tokT = nc.dram_tensor("x_tokT", [D, tok], F32, kind="ExternalInput")
    w_sb_h = nc.dram_tensor("w_chunks", [128, KC * 128], F32, kind="ExternalInput")
    aug_uv = nc.dram_tensor("aug_uv", [3, 128], F32, kind="ExternalInput")
    aug_tok = nc.dram_tensor("aug_tok", [3, tok], F32, kind="ExternalInput")
    chain_perm = nc.dram_tensor("chain_perm", [128, VL * 128], F32, kind="ExternalInput")
    pi_tab = nc.dram_tensor("pi_tab", [128, 128], F32, kind="ExternalInput")
    u_iota = nc.dram_tensor("u_iota", [128, 128], F32, kind="ExternalInput")
    ident = nc.dram_tensor("ident", [128, 128], F32, kind="ExternalInput")
    cfdec = nc.dram_tensor("cfdec", [128, VL], F32, kind="ExternalInput")
    v_iota = nc.dram_tensor("v_iota", [128, VL], F32, kind="ExternalInput")
    vh8u = nc.dram_tensor("vh8u", [128, 1], U16, kind="ExternalInput")
    mask_rep = nc.dram_tensor("mask_rep", [128, t_steps], U8, kind="ExternalInput")
    nmask_rep = nc.dram_tensor("nmask_rep", [128, t_steps], U8, kind="ExternalInput")

    out_potT = nc.dram_tensor("out_potT", [U, tok], F32, kind="ExternalOutput")
    out_oh = nc.dram_tensor("out_oh", [tok, U], F32, kind="ExternalOutput")

    with tile.TileContext(nc) as tc, ExitStack() as ctx:
        persist = ctx.enter_context(tc.tile_pool(name="persist", bufs=1))

        # ---- persistent SBUF tiles ----
        w_sb = persist.tile([128, KC * 128], F32, tag="w_sb")
        aug_uv_sb = persist.tile([3, 128], F32, tag="aug_uv")
        aug_tok_sb = persist.tile([3, BL, t_steps], F32, tag="aug_tok")
        chain_sb = persist.tile([128, VL, 128], F32, tag="chain")
        pi_sb = persist.tile([128, 128], F32, tag="pi")
        uio_sb = persist.tile([128, 128], F32, tag="uio")
        ident_sb = persist.tile([128, 128], F32, tag="ident")
        cfdec_sb = persist.tile([128, VL], F32, tag="cfdec")
        vio_sb = persist.tile([128, VL], F32, tag="vio")
        vh8u_sb = persist.tile([128, 1], U16, tag="vh8u")
        mask_sb = persist.tile([128, t_steps], U8, tag="mask")
        nmask_sb = persist.tile([128, t_steps], U8, tag="nmask")
        pot_rep = persist.tile([128, VL, t_steps], F32, tag="pot_rep")
        state_row = persist.tile([128, 128], F32, tag="state_row")
        bp_raw = persist.tile([128, t_steps, VL], U16, tag="bp_raw")
        tag_store = persist.tile([128, t_steps], F32, tag="tag_store")

        nc.sync.dma_start(w_sb[:], w_sb_h.ap())
        nc.sync.dma_start(aug_uv_sb[:], aug_uv.ap())
        nc.sync.dma_start(
            aug_tok_sb[:].rearrange("p a b -> p (a b)"), aug_tok.ap())
        nc.sync.dma_start(chain_sb[:].rearrange("p a b -> p (a b)"), chain_perm.ap())
        nc.sync.dma_start(pi_sb[:], pi_tab.ap())
        nc.sync.dma_start(uio_sb[:], u_iota.ap())
        nc.sync.dma_start(ident_sb[:], ident.ap())
        nc.sync.dma_start(cfdec_sb[:], cfdec.ap())
        nc.sync.dma_start(vio_sb[:], v_iota.ap())
        nc.sync.dma_start(vh8u_sb[:], vh8u.ap())
        nc.sync.dma_start(mask_sb[:], mask_rep.ap())
        nc.sync.dma_start(nmask_sb[:], nmask_rep.ap())

        # ================= Phase 1: potentials matmuls =================
        # t-major slabs so pot_rep fills progressively and the Viterbi loop can
        # start as soon as slab 0 lands (phase 1 hides under phase 2).
        tsl = min(128, t_steps)             # t-steps per slab
        n_sl = t_steps // tsl
        bh = max(1, tch // tsl)             # sequences per psA half
        n_h = BL // bh
        assert bh * tsl <= tch and t_steps % tsl == 0
        with tc.tile_pool(name="ph1", bufs=2) as ph1, \
             tc.tile_pool(name="ph1o", bufs=2) as ph1o, \
             tc.tile_pool(name="psA", bufs=2, space="PSUM") as psA_pool:
            for sl in range(n_sl):
                c0 = sl * tsl
                for h in range(n_h):
                    b0 = h * bh
                    cols = bh * tsl
                    xT = ph1.tile([128, KC, cols], F32, tag="xT")
                    for k in range(KC):
                        nc.sync.dma_start(
                            xT[:, k, :].rearrange("p (a b) -> p a b", a=bh),
                            bass.AP(x_tokT, k * 128 * tok + b0 * t_steps + c0,
                                    [[tok, 128], [t_steps, bh], [1, tsl]]),
                        )
                    psA = psA_pool.tile([128, cols], F32, tag="psA")
                    for k in range(KC):
                        nc.tensor.matmul(
                            psA[:], w_sb[:, k * 128:(k + 1) * 128], xT[:, k, :],
                            start=(k == 0), stop=False,
                        )
                    nc.tensor.matmul(
                        psA[:], aug_uv_sb[:],
                        aug_tok_sb[:, b0:b0 + bh, c0:c0 + tsl],
                        start=False, stop=True,
                    )
                    po = ph1o.tile([128, cols], F32, tag="po")
                    nc.vector.tensor_copy(po[:], psA[:])
                    nc.sync.dma_start(
                        bass.AP(out_potT, b0 * t_steps + c0,
                                [[tok, 128], [t_steps, bh], [1, tsl]]),
                        po[:],
                    )
                # potT (DRAM) -> pot_rep[(b,vh), vl, t] = pot[b, t, 8*vh+vl]
                for b in range(BL):
                    nc.sync.dma_start(
                        pot_rep[b * VH:(b + 1) * VH, :, c0:c0 + tsl],
                        bass.AP(out_potT, b * t_steps + c0,
                                [[VL * tok, VH], [tok, VL], [1, tsl]]),
                    )

        # ================= Phase 2: forward Viterbi =================
        nc.vector.tensor_copy(state_row[:, 0:VL], pot_rep[:, :, 0])
        nc.vector.memset(bp_raw[:, 0, :], 0)

        masks = {s: _shuffle_mask(s) for s in (1, 2, 4, 8)}
        sc_pool = ctx.enter_context(tc.tile_pool(name="sc", bufs=2))
        fwd = ctx.enter_context(tc.tile_pool(name="fwd", bufs=2))
        prev = None  # (sc, cmax, t) awaiting backpointer extraction
        for t in range(1, t_steps):
            for s in (1, 2, 4, 8):
                nc.vector.stream_shuffle(
                    state_row[:, 8 * s:16 * s], state_row[:, 0:8 * s], masks[s]
                )
            sc = sc_pool.tile([128, VL, 128], F32, tag="sc")
            srow_b = state_row[:].unsqueeze(1)
            if add_split > 0:
                nc.gpsimd.tensor_tensor(
                    sc[:, add_split:, :], chain_sb[:, add_split:, :],
                    srow_b.broadcast_to([128, VL - add_split, 128]), ALU.add,
                )
                nc.vector.tensor_tensor(
                    sc[:, 0:add_split, :], chain_sb[:, 0:add_split, :],
                    srow_b.broadcast_to([128, add_split, 128]), ALU.add,
                )
            else:
                nc.vector.tensor_tensor(
                    sc[:], chain_sb[:], srow_b.broadcast_to([128, VL, 128]), ALU.add
                )
            # deferred backpointer extraction for the previous step (keeps the
            # DVE busy while GpSimd/DVE finish this step's adds)
            if prev is not None:
                psc, pcm, pt = prev
                nc.vector.max_index(
                    bp_raw[:, pt, :], pcm[:],
                    psc[:].rearrange("p a b -> p (a b)"),
                )
            cmax = fwd.tile([128, VL], F32, tag="cmax")
            nc.vector.tensor_reduce(cmax[:], sc[:], AX.X, ALU.max)
            tmp = fwd.tile([128, VL], F32, tag="tmp")
            nc.vector.tensor_tensor(tmp[:], cmax[:], pot_rep[:, :, t], ALU.add)
            nc.vector.copy_predicated(
                state_row[:, 0:VL],
                mask_sb[:, t:t + 1].broadcast_to([128, VL]), tmp[:],
            )
            prev = (sc, cmax, t)
        psc, pcm, pt = prev
        nc.vector.max_index(
            bp_raw[:, pt, :], pcm[:],
            psc[:].rearrange("p a b -> p (a b)"),
        )

        # ---- final tag: argmax of the (mask-frozen) final state ----
        bwd = ctx.enter_context(tc.tile_pool(name="bwd", bufs=1))
        for s in (1, 2, 4, 8):
            nc.vector.stream_shuffle(
                state_row[:, 8 * s:16 * s], state_row[:, 0:8 * s], masks[s]
            )
        top8 = bwd.tile([128, 8], F32, tag="top8")
        nc.vector.max(top8[:], state_row[:])
        idx8 = bwd.tile([128, 8], U16, tag="idx8")
        nc.vector.max_index(idx8[:], top8[:], state_row[:])
        lt1 = bwd.tile([128, 1], U16, tag="lt1")
        nc.vector.tensor_tensor(lt1[:], idx8[:, 0:1], vh8u_sb[:], ALU.add)
        lt2 = bwd.tile([128, 1], U16, tag="lt2")
        nc.vector.tensor_scalar(lt2[:], lt1[:], 127, None, ALU.bitwise_and)
        nc.vector.tensor_copy(tag_store[:, t_steps - 1:t_steps], lt2[:])

        # ================= Phase 3: backward trace =================
        ring_pool = ctx.enter_context(tc.tile_pool(name="ring", bufs=2))
        dec_pool = ctx.enter_context(tc.tile_pool(name="dec", bufs=2))
        sel_pool = ctx.enter_context(tc.tile_pool(name="sel", bufs=2))
        oh_pool = ctx.enter_context(tc.tile_pool(name="oh", bufs=3))
        tt_pool = ctx.enter_context(tc.tile_pool(name="tt", bufs=2))
        psT_pool = ctx.enter_context(tc.tile_pool(name="psT", bufs=2, space="PSUM"))
        blk = (128 // ch) if ch < 128 else 1   # chunks per one-hot block

        def emit_onehot_block(c0, clen):
            psT = psT_pool.tile([clen, 128], F32, tag="psT")
            nc.tensor.matmul(
                psT[:], tag_store[:, c0:c0 + clen], ident_sb[:],
                start=True, stop=True,
            )
            tagT = tt_pool.tile([128, VL], F32, tag="tagT")
            nc.vector.tensor_copy(tagT[0:clen, :], psT[:, 0:128:VH])
            for b in range(BL):
                oh_t = oh_pool.tile([128, 128], F32, tag="oh")
                nc.vector.tensor_tensor(
                    oh_t[0:clen, :], uio_sb[0:clen, :],
                    tagT[0:clen, b:b + 1].broadcast_to([clen, 128]),
                    ALU.is_equal,
                )
                nc.sync.dma_start(
                    bass.AP(out_oh, (b * t_steps + c0) * U, [[U, clen], [1, U]]),
                    oh_t[0:clen, :],
                )

        for c in range(n_ch - 1, -1, -1):
            tb = c * ch
            # decode raw indices for this chunk: u = (idx + 8*vh - 128*vl),
            # minus 128 when >= 128 (value known < 256)
            scrA = dec_pool.tile([128, ch, VL], F32, tag="scrA")
            scrB = dec_pool.tile([128, ch, VL], F32, tag="scrB")
            nc.vector.tensor_copy(scrA[:], bp_raw[:, tb:tb + ch, :])
            nc.vector.tensor_tensor(
                scrB[:], scrA[:],
                cfdec_sb[:].unsqueeze(1).broadcast_to([128, ch, VL]), ALU.add,
            )
            nc.vector.tensor_scalar(scrA[:], scrB[:], 128.0, None, ALU.is_ge)
            ring = ring_pool.tile([128, ch, 128], F32, tag="bpring")
            nc.vector.scalar_tensor_tensor(
                ring[:, :, 0:VL], scrA[:], -128.0, scrB[:], ALU.mult, ALU.add
            )
            nc.vector.copy_predicated(
                ring[:, :, 0:VL],
                nmask_sb[:, tb:tb + ch].unsqueeze(2).broadcast_to([128, ch, VL]),
                vio_sb[:].unsqueeze(1).broadcast_to([128, ch, VL]),
            )
            for s in (1, 2, 4, 8):
                nc.vector.stream_shuffle(
                    ring[:, :, 8 * s:16 * s], ring[:, :, 0:8 * s], masks[s]
                )
            t_lo = max(tb, 1)
            for t in range(tb + ch - 1, t_lo - 1, -1):
                sel = sel_pool.tile([128, 128], F32, tag="sel")
                nc.vector.scalar_tensor_tensor(
                    sel[:], pi_sb[:], tag_store[:, t:t + 1], ring[:, t - tb, :],
                    ALU.is_equal, ALU.mult, accum_out=tag_store[:, t - 1:t],
                )
            if c % blk == 0:
                c0 = (c // blk) * min(128, t_steps)
                emit_onehot_block(c0, min(128, t_steps - c0))

    nc.compile()
    if not nc.is_finalized():
        nc.finalize()
    return nc


def _host_prep(inputs, mask, kern, bias, chain_kernel, left_b, right_b, t_steps=T):
    """Build per-core input maps (all numpy, float32)."""
    tok = BL * t_steps
    p = np.arange(128)
    vh = p % VH
    j = np.arange(128)
    # pi[p, j] = (8*vh + j) mod 128 : state/bp all-gather order per partition
    pi = (8 * vh[:, None] + j[None, :]) % 128
    v_of_p = vh[:, None] * VL + np.arange(VL)[None, :]  # [128, VL]

    chain_pp = np.empty((128, VL, 128), np.float32)
    for pp in range(128):
        chain_pp[pp] = chain_kernel[pi[pp]][:, v_of_p[pp]].T  # [VL, 128]

    w_chunks = kern.reshape(KC, 128, 128).transpose(1, 0, 2).reshape(128, KC * 128)
    aug_uv = np.stack([bias, left_b, right_b]).astype(np.float32)

    lengths = mask.sum(axis=1).astype(np.int64)
    n_cores = inputs.shape[0] // BL
    in_maps = []
    for c in range(n_cores):
        bs = c * BL
        xl = inputs[bs:bs + BL].reshape(tok, D)
        ones = np.ones(tok, np.float32)
        start01 = np.zeros((BL, t_steps), np.float32)
        end01 = np.zeros((BL, t_steps), np.float32)
        start01[:, 0] = 1.0
        for b in range(BL):
            end01[b, lengths[bs + b] - 1] = 1.0
        m = {
            "x_tokT": np.ascontiguousarray(xl.T),
            "w_chunks": np.ascontiguousarray(w_chunks),
            "aug_uv": np.ascontiguousarray(aug_uv),
            "aug_tok": np.ascontiguousarray(
                np.stack([ones, start01.ravel(), end01.ravel()])),
            "chain_perm": np.ascontiguousarray(chain_pp.reshape(128, VL * 128)),
            "pi_tab": pi.astype(np.float32),
            "u_iota": np.tile(j.astype(np.float32), (128, 1)),
            "ident": np.eye(128, dtype=np.float32),
            "cfdec": (8.0 * vh[:, None] - 128.0 * np.arange(VL)[None, :]
                      ).astype(np.float32),
            "v_iota": v_of_p.astype(np.float32),
            "vh8u": (8 * vh[:, None]).astype(np.uint16),
            "mask_rep": mask[bs + p // VH, :t_steps].astype(np.uint8),
            "nmask_rep": (~mask[bs + p // VH, :t_steps]).astype(np.uint8),
        }
        in_maps.append(m)
    return in_maps


_NC_CACHE = {}


def kernel(inputs, mask, kernel, bias, chain_kernel, left_boundary, right_boundary):
    inputs = np.asarray(inputs, np.float32)
    mask_np = np.asarray(mask)
    kern = np.asarray(kernel, np.float32)
    bias = np.asarray(bias, np.float32)
    chain = np.asarray(chain_kernel, np.float32)
    lb = np.asarray(left_boundary, np.float32)
    rb = np.asarray(right_boundary, np.float32)

    from concourse.bass_utils import run_bass_kernel_spmd

    add_split = int(os.environ.get("KERNEL_ADD_SPLIT", "0"))
    key = ("nc", add_split)
    if key not in _NC_CACHE:
        _NC_CACHE[key] = build_module(add_split=add_split)
    nc = _NC_CACHE[key]

    in_maps = _host_prep(inputs, mask_np, kern, bias, chain, lb, rb)
    res = run_bass_kernel_spmd(
        nc, in_maps, core_ids=list(range(NCORES)),
        trace=bool(int(os.environ.get("KERNEL_TRACE", "0"))),
    )
    pot = np.concatenate(
        [np.ascontiguousarray(r["out_potT"].reshape(U, BL, T).transpose(1, 2, 0))
         for r in res.results], axis=0)
    oh = np.concatenate(
        [r["out_oh"].reshape(BL, T, U) for r in res.results], axis=0)
    if res.exec_time_ns is not None:
        print(f"HW exec time: {res.exec_time_ns} ns")
    return pot, oh
